# revision 25
# baseline (speedup 1.0000x reference)
"""Additive (Bahdanau) attention kernel for Trainium2, 8 NeuronCores.

Math (per batch b):
  Wv = v @ W            [Tv, D]
  Uh = h @ U            [Th, D]
  q[s,t] = sum_d w[d] * tanh(Uh[s,d] + Wv[t,d] + b[d])
  beta = softmax_t(q)
  u = beta @ v          [Th, F]

Sharding: pure data-parallel over B (16 batches -> 2 per core), weights
replicated. No collectives.

Per-core layout strategy: the broadcast-add (Uh[s,d] + Wv[t,d]) is built with
D on partitions (2 chunks of 128) so BOTH operands broadcast along stride-0
FREE dims (no partition broadcast needed).  tanh on ScalarE (the only
transcendental engine -> critical path ~29us/core).  The w-dot contraction
over d runs on TensorE with a 32-row-replicated w as the stationary operand +
4-way column tiling, so q lands in PSUM across partitions 0..127 and can be
drained full-lane, then reshaped to [s, t] with one SBUF->SBUF DMA that
exploits the row redundancy.  Softmax on [s=64, t=128] (VectorE reductions +
ScalarE exp with fused accumulated denominator), context matmul on TensorE.
"""

import numpy as np

B, TV, TH, F, H, D = 16, 128, 64, 512, 512, 256
NCORES = 8
BL = B // NCORES  # 2 batches per core
DCN = 2  # d chunks of 128
FCN = 4  # f chunks of 128
HCN = 4  # h chunks of 128

_CACHE = {}


def _split_excess_waits(nc, mybir):
    """The per-engine ISA instruction structs encode a single sync-wait
    command, but Tile sometimes attaches 2-3 waits to one instruction, which
    walrus rejects ("Too many sync wait commands").  Split: keep one wait on
    the instruction and insert same-engine NoOp carriers (one wait each)
    immediately before it."""
    EXEMPT = ("InstUnconditionalBranch", "InstCall")
    k = 0
    for f in nc.m.functions:
        for blk in f.blocks:
            insts = list(blk.instructions)
            out, changed = [], False
            for inst in insts:
                si = inst.sync_info
                tn = type(inst).__name__
                if (si is not None and si.on_wait and len(si.on_wait) > 1
                        and tn not in EXEMPT):
                    waits = list(si.on_wait)
                    for wext in waits[:-1]:
                        noop = mybir.InstNoOp(name=f"wsplit-{k}")
                        k += 1
                        noop.engine = inst.engine
                        noop.sync_info = mybir.SyncInfo(
                            on_wait=[wext], on_update=[]
                        )
                        out.append(noop)
                    inst.sync_info = mybir.SyncInfo(
                        on_wait=waits[-1:], on_update=list(si.on_update or [])
                    )
                    changed = True
                out.append(inst)
            if changed:
                blk.instructions = out


def _build_nc():
    import concourse.bass as bass
    import concourse.tile as tile
    from concourse import mybir
    from concourse.masks import make_identity

    f32 = mybir.dt.float32
    bf16 = mybir.dt.bfloat16
    AF = mybir.ActivationFunctionType
    AX = mybir.AxisListType

    nc = bass.Bass()
    v_e = nc.declare_dram_parameter("v", [BL, TV, F], f32, isOutput=False)
    h_e = nc.declare_dram_parameter("h", [BL, TH, H], f32, isOutput=False)
    W_e = nc.declare_dram_parameter("W", [F, D], f32, isOutput=False)
    U_e = nc.declare_dram_parameter("U", [H, D], f32, isOutput=False)
    b_e = nc.declare_dram_parameter("b", [D], f32, isOutput=False)
    w_e = nc.declare_dram_parameter("w", [D, 1], f32, isOutput=False)
    out_e = nc.declare_dram_parameter("out", [BL, TH, F], f32, isOutput=True)

    with tile.TileContext(nc) as tc:
        with (
            tc.tile_pool(name="consts", bufs=1) as consts,
            tc.tile_pool(name="sbig", bufs=3) as spool,
            tc.tile_pool(name="fbig", bufs=4) as fpool,
            tc.tile_pool(name="qred", bufs=2) as qredp,
            tc.tile_pool(name="smalls", bufs=2) as smalls,
            tc.tile_pool(name="ps_t", bufs=2, space="PSUM") as ps_t,
            tc.tile_pool(name="ps_p", bufs=2, space="PSUM") as ps_p,
            tc.tile_pool(name="ps_u", bufs=1, space="PSUM") as ps_u,
            tc.tile_pool(name="ps_q", bufs=3, space="PSUM") as ps_q,
        ):
            # ---------- load + prep ----------
            # Chunked loads: each HW DMA queue moves ~78 GB/s, so big tensors
            # are split across queues and casts run per-chunk as data lands.
            # v/h on sync, W/U on scalar, b/w on gpsimd (SWDGE).
            vf32 = consts.tile([128, BL, F], f32)
            hf32 = consts.tile([TH, BL, H], f32)
            Wf32 = consts.tile([128, FCN, D], f32)
            Uf32 = consts.tile([128, HCN, D], f32)
            for fc in range(FCN):  # batch-0 v in 64KB chunks (critical path)
                nc.sync.dma_start(
                    out=vf32[:, 0, fc * 128 : (fc + 1) * 128],
                    in_=v_e[0, :, fc * 128 : (fc + 1) * 128],
                )
            nc.sync.dma_start(out=hf32[:, 0, :], in_=h_e[0])
            nc.sync.dma_start(out=vf32[:, 1, :], in_=v_e[1])
            nc.sync.dma_start(out=hf32[:, 1, :], in_=h_e[1])
            for fc in range(FCN):
                nc.scalar.dma_start(
                    out=Wf32[:, fc, :], in_=W_e[fc * 128 : (fc + 1) * 128, :]
                )
            for hc in range(HCN):
                nc.scalar.dma_start(
                    out=Uf32[:, hc, :], in_=U_e[hc * 128 : (hc + 1) * 128, :]
                )
            bsb = consts.tile([128, DCN], f32)
            nc.gpsimd.dma_start(out=bsb[:], in_=b_e[:].rearrange("(c p) -> p c", p=128))
            wsb = consts.tile([128, DCN], f32)
            nc.gpsimd.dma_start(out=wsb[:], in_=w_e[:, 0].rearrange("(c p) -> p c", p=128))

            vbf = consts.tile([128, BL, F], bf16)
            hbf = consts.tile([TH, BL, H], bf16)
            Wbf = consts.tile([128, FCN, D], bf16)
            Ubf = consts.tile([128, HCN, D], bf16)
            for fc in range(FCN):
                nc.vector.tensor_copy(
                    vbf[:, 0, fc * 128 : (fc + 1) * 128],
                    vf32[:, 0, fc * 128 : (fc + 1) * 128],
                )
                nc.scalar.copy(Wbf[:, fc, :], Wf32[:, fc, :])
            for hc in range(HCN):
                nc.vector.tensor_copy(
                    hbf[:, 0, hc * 128 : (hc + 1) * 128],
                    hf32[:, 0, hc * 128 : (hc + 1) * 128],
                )
                nc.vector.tensor_copy(Ubf[:, hc, :], Uf32[:, hc, :])
            wbf = consts.tile([128, DCN], bf16)
            nc.vector.tensor_copy(wbf[:], wsb[:])
            w_rep = consts.tile([128, DCN, 32], bf16)
            for dc in range(DCN):
                nc.vector.tensor_copy(
                    w_rep[:, dc, :], wbf[:, dc : dc + 1].broadcast_to([128, 32])
                )

            ident = consts.tile([128, 128], bf16)
            make_identity(nc, ident)

            # transposes via PE (f32 in, bf16 out through the PSUM copyback)
            vT = consts.tile([128, BL, FCN, 128], bf16)   # [f_p, b, fc, t]
            hT = consts.tile([128, BL, HCN, TH], bf16)    # [h_p, b, hc, s]
            WvT = consts.tile([128, BL, DCN, TV], bf16)   # [d_p, b, dc, t]
            Uh2 = consts.tile([128, BL, DCN, TH, 2], bf16)  # [d_p, b, dc, s, dup]

            for b in range(BL):
                # ---------- transposes + projections ----------
                if b == 0:
                    # critical prefix: PE transposes interleaved with the
                    # accumulating projection matmuls (all bf16)
                    wv_ps = [ps_p.tile([128, 128], f32, tag="psp", name=f"wv_ps{dc}") for dc in range(DCN)]
                    for fc in range(FCN):
                        tp = ps_t.tile([128, 128], bf16, tag="pst")
                        nc.tensor.transpose(
                            tp[:], vbf[:, b, fc * 128 : (fc + 1) * 128], ident[:]
                        )
                        nc.vector.tensor_copy(vT[:, b, fc, :], tp[:])
                        for dc in range(DCN):
                            nc.tensor.matmul(
                                wv_ps[dc][:],
                                lhsT=Wbf[:, fc, dc * 128 : (dc + 1) * 128],
                                rhs=vT[:, b, fc, :],
                                start=(fc == 0),
                                stop=(fc == FCN - 1),
                            )
                    for dc in range(DCN):
                        nc.vector.tensor_copy(WvT[:, b, dc, :], wv_ps[dc][:])
                    uh_ps = [ps_p.tile([128, TH], f32, tag="psp", name=f"uh_ps{dc}") for dc in range(DCN)]
                    for hc in range(HCN):
                        tp = ps_t.tile([128, 128], bf16, tag="pst")
                        nc.tensor.transpose(
                            tp[:, :TH],
                            hbf[:, b, hc * 128 : (hc + 1) * 128],
                            ident[:TH, :TH],
                        )
                        nc.vector.tensor_copy(hT[:, b, hc, :], tp[:, :TH])
                        for dc in range(DCN):
                            nc.tensor.matmul(
                                uh_ps[dc][:],
                                lhsT=Ubf[:, hc, dc * 128 : (dc + 1) * 128],
                                rhs=hT[:, b, hc, :],
                                start=(hc == 0),
                                stop=(hc == HCN - 1),
                            )
                    for dc in range(DCN):
                        # duplicate each s value twice along free so the later
                        # tensor_tensor read has innermost step 1 (2x DVE mode)
                        nc.vector.tensor_copy(
                            Uh2[:, b, dc, :, :],
                            uh_ps[dc][:].unsqueeze(2).broadcast_to([128, TH, 2]),
                        )
                else:
                    # overlapped under batch-0 compute: xbar DMA transposes on
                    # the otherwise-idle sync engine
                    nc.vector.tensor_copy(vbf[:, b, :], vf32[:, b, :])
                    nc.vector.tensor_copy(hbf[:, b, :], hf32[:, b, :])
                    for fc in range(FCN):
                        nc.sync.dma_start_transpose(
                            vT[:, b, fc, :], vbf[:, b, fc * 128 : (fc + 1) * 128]
                        )
                    for hc in range(HCN):
                        nc.sync.dma_start_transpose(
                            hT[:, b, hc, :], hbf[:, b, hc * 128 : (hc + 1) * 128]
                        )
                    for dc in range(DCN):
                        dlo, dhi = dc * 128, (dc + 1) * 128
                        wv_ps0 = ps_p.tile([128, 128], f32, tag="psp")
                        for fc in range(FCN):
                            nc.tensor.matmul(
                                wv_ps0[:],
                                lhsT=Wbf[:, fc, dlo:dhi],
                                rhs=vT[:, b, fc, :],
                                start=(fc == 0),
                                stop=(fc == FCN - 1),
                            )
                        nc.vector.tensor_copy(WvT[:, b, dc, :], wv_ps0[:])
                        uh_ps0 = ps_p.tile([128, TH], f32, tag="psp")
                        for hc in range(HCN):
                            nc.tensor.matmul(
                                uh_ps0[:],
                                lhsT=Ubf[:, hc, dlo:dhi],
                                rhs=hT[:, b, hc, :],
                                start=(hc == 0),
                                stop=(hc == HCN - 1),
                            )
                        nc.vector.tensor_copy(
                            Uh2[:, b, dc, :, :],
                            uh_ps0[:].unsqueeze(2).broadcast_to([128, TH, 2]),
                        )

                # ---------- main: S build (DVE) -> tanh (ACT) -> w-dot (PE) ----
                # per s-half: col-group g covers s_local in [8g, 8g+8)
                for sh in range(2):  # s-halves of 32
                    f_tiles = {}
                    qps = [ps_q.tile([128, 512], f32, tag="qps", name=f"qps{R}")
                           for R in range(2)]
                    for dc in range(DCN):
                        s_t = spool.tile([128, 32, 128], bf16, tag="s")
                        in0 = WvT[:, b, dc, :].unsqueeze(1).broadcast_to([128, 32, 128])
                        in1 = (
                            Uh2[:, b, dc, sh * 32 : (sh + 1) * 32, :]
                            .unsqueeze(2)
                            .broadcast_to([128, 32, 64, 2])
                        )
                        nc.vector.tensor_add(s_t[:], in0, in1)
                        f_t = fpool.tile([128, 32, 128], bf16, tag="f")
                        nc.scalar.activation(
                            f_t[:], s_t[:], AF.Tanh, bias=bsb[:, dc : dc + 1], scale=1.0
                        )
                        f_tiles[dc] = f_t
                        # w-dot for this d-chunk: q[s,t] = sum_d w[d] f[d,s,t]
                        # s_local = 16R + 4g + s_sub  (R-blocks are contiguous)
                        for R in range(2):
                            for g in range(4):
                                so = 16 * R + 4 * g
                                nc.tensor.matmul(
                                    qps[R][32 * g : 32 * (g + 1), :],
                                    lhsT=w_rep[:, dc, :],
                                    rhs=f_t[:, so : so + 4, :],
                                    start=(dc == 0),
                                    stop=(dc == DCN - 1),
                                    tile_position=(0, 32 * g),
                                )
                    qred = qredp.tile([128, 2, 512], bf16, tag="qred")
                    eT = smalls.tile([128, 32], bf16, tag="eT")
                    for R in range(2):
                        rlo, rhi = 16 * R, 16 * (R + 1)
                        e_bf = smalls.tile([16, TV], bf16, tag="e", name=f"e{R}")
                        den = smalls.tile([16, 1], f32, tag="den", name=f"den{R}")
                        rden = smalls.tile([16, 1], f32, tag="rden", name=f"rden{R}")
                        q_sb = qredp.tile([16, TV], bf16, tag="qsb", name=f"qsb{R}")
                        nc.vector.tensor_copy(qred[:, R, :], qps[R][:])
                        # row 32g holds q for s_local 16R+[4g,4g+4): diagonal
                        nc.sync.dma_start(
                            out=q_sb[:],
                            in_=qred[::32, R, :].rearrange("g (s t) -> g s t", s=4),
                        )
                        # softmax over t with fused denominator accumulation
                        nc.scalar.activation(
                            e_bf[:], q_sb[:], AF.Exp,
                            bias=0.0, scale=1.0, accum_out=den[:],
                        )
                        nc.vector.reciprocal(rden[:], den[:])
                        # context: u = (e @ v) / den
                        btp = ps_t.tile([128, 16], bf16, tag="pst")
                        nc.tensor.transpose(btp[:], e_bf[:], ident[:16, :16])
                        nc.vector.tensor_copy(eT[:, rlo:rhi], btp[:])
                        ups = ps_u.tile([16, F], f32)
                        nc.tensor.matmul(
                            ups[:], lhsT=eT[:, rlo:rhi], rhs=vbf[:, b, :],
                            start=True, stop=True,
                        )
                        usb = smalls.tile([16, F], f32, tag="usb")
                        nc.vector.tensor_scalar_mul(usb[:], ups[:], rden[:])
                        nc.sync.dma_start(
                            out=out_e[b, sh * 32 + rlo : sh * 32 + rhi, :],
                            in_=usb[:],
                        )

    _split_excess_waits(nc, mybir)
    return nc


def _get_nc():
    if "nc" not in _CACHE:
        _CACHE["nc"] = _build_nc()
    return _CACHE["nc"]


def _in_maps(v, h, W, U, b, w):
    v = np.ascontiguousarray(np.asarray(v, dtype=np.float32))
    h = np.ascontiguousarray(np.asarray(h, dtype=np.float32))
    W = np.ascontiguousarray(np.asarray(W, dtype=np.float32))
    U = np.ascontiguousarray(np.asarray(U, dtype=np.float32))
    b = np.ascontiguousarray(np.asarray(b, dtype=np.float32))
    w = np.ascontiguousarray(np.asarray(w, dtype=np.float32))
    return [
        {
            "v": np.ascontiguousarray(v[i * BL : (i + 1) * BL]),
            "h": np.ascontiguousarray(h[i * BL : (i + 1) * BL]),
            "W": W,
            "U": U,
            "b": b,
            "w": w,
        }
        for i in range(NCORES)
    ]


def _run(in_maps, trace=False, tmpdir=None):
    from concourse.bass_utils import run_bass_kernel_spmd

    nc = _get_nc()
    return run_bass_kernel_spmd(
        nc, in_maps, core_ids=list(range(NCORES)), trace=trace, tmpdir=tmpdir
    )


def kernel(v, h, W, U, b, w):
    res = _run(_in_maps(v, h, W, U, b, w), trace=False)
    return np.concatenate([res.results[i]["out"] for i in range(NCORES)], axis=0)


def _install_ntff_hook():
    """The agent image's antenv lacks axon_hooks; recreate it so
    run_bass_kernel_spmd(trace=True) can NTFF-profile via the axon .so."""
    import sys
    import types

    try:
        from antenv.axon_hooks import get_axon_ntff_profile_hook  # noqa: F401
        return
    except ImportError:
        pass
    import antenv
    from trn_agent_boot.trn_boot import _ntff_profile_via_ctypes

    mod = types.ModuleType("antenv.axon_hooks")
    state = {"hook": _ntff_profile_via_ctypes("/opt/axon/libaxon_pjrt.so")}
    mod.set_axon_ntff_profile_hook = lambda h: state.__setitem__("hook", h)
    mod.get_axon_ntff_profile_hook = lambda: state["hook"]
    sys.modules["antenv.axon_hooks"] = mod
    antenv.axon_hooks = mod


def kernel_traced(v, h, W, U, b, w, tmpdir=None):
    """Returns (output, exec_time_ns) using the NTFF profile path."""
    _install_ntff_hook()
    import concourse.bass_utils as bu

    bu.upload_artifacts = lambda d: str(d)  # keep artifacts local
    res = _run(_in_maps(v, h, W, U, b, w), trace=True, tmpdir=tmpdir)
    out = np.concatenate([res.results[i]["out"] for i in range(NCORES)], axis=0)
    return out, res.exec_time_ns


# revision 26
# speedup vs baseline: 1.0076x; 1.0076x over previous
"""Additive (Bahdanau) attention kernel for Trainium2, 8 NeuronCores.

Math (per batch b):
  Wv = v @ W            [Tv, D]
  Uh = h @ U            [Th, D]
  q[s,t] = sum_d w[d] * tanh(Uh[s,d] + Wv[t,d] + b[d])
  beta = softmax_t(q)
  u = beta @ v          [Th, F]

Sharding: pure data-parallel over B (16 batches -> 2 per core), weights
replicated. No collectives.

Per-core layout strategy: the broadcast-add (Uh[s,d] + Wv[t,d]) is built with
D on partitions (2 chunks of 128) so BOTH operands broadcast along stride-0
FREE dims (no partition broadcast needed).  tanh on ScalarE (the only
transcendental engine -> critical path ~29us/core).  The w-dot contraction
over d runs on TensorE with a 32-row-replicated w as the stationary operand +
4-way column tiling, so q lands in PSUM across partitions 0..127 and can be
drained full-lane, then reshaped to [s, t] with one SBUF->SBUF DMA that
exploits the row redundancy.  Softmax on [s=64, t=128] (VectorE reductions +
ScalarE exp with fused accumulated denominator), context matmul on TensorE.
"""

import numpy as np

B, TV, TH, F, H, D = 16, 128, 64, 512, 512, 256
NCORES = 8
BL = B // NCORES  # 2 batches per core
DCN = 2  # d chunks of 128
FCN = 4  # f chunks of 128
HCN = 4  # h chunks of 128

_CACHE = {}


def _split_excess_waits(nc, mybir):
    """The per-engine ISA instruction structs encode a single sync-wait
    command, but Tile sometimes attaches 2-3 waits to one instruction, which
    walrus rejects ("Too many sync wait commands").  Split: keep one wait on
    the instruction and insert same-engine NoOp carriers (one wait each)
    immediately before it."""
    EXEMPT = ("InstUnconditionalBranch", "InstCall")
    k = 0
    for f in nc.m.functions:
        for blk in f.blocks:
            insts = list(blk.instructions)
            out, changed = [], False
            for inst in insts:
                si = inst.sync_info
                tn = type(inst).__name__
                if (si is not None and si.on_wait and len(si.on_wait) > 1
                        and tn not in EXEMPT):
                    waits = list(si.on_wait)
                    for wext in waits[:-1]:
                        noop = mybir.InstNoOp(name=f"wsplit-{k}")
                        k += 1
                        noop.engine = inst.engine
                        noop.sync_info = mybir.SyncInfo(
                            on_wait=[wext], on_update=[]
                        )
                        out.append(noop)
                    inst.sync_info = mybir.SyncInfo(
                        on_wait=waits[-1:], on_update=list(si.on_update or [])
                    )
                    changed = True
                out.append(inst)
            if changed:
                blk.instructions = out


def _build_nc():
    import concourse.bass as bass
    import concourse.tile as tile
    from concourse import mybir
    from concourse.masks import make_identity

    f32 = mybir.dt.float32
    bf16 = mybir.dt.bfloat16
    AF = mybir.ActivationFunctionType
    AX = mybir.AxisListType

    nc = bass.Bass()
    v_e = nc.declare_dram_parameter("v", [BL, TV, F], f32, isOutput=False)
    h_e = nc.declare_dram_parameter("h", [BL, TH, H], f32, isOutput=False)
    W_e = nc.declare_dram_parameter("W", [F, D], f32, isOutput=False)
    U_e = nc.declare_dram_parameter("U", [H, D], f32, isOutput=False)
    b_e = nc.declare_dram_parameter("b", [D], f32, isOutput=False)
    w_e = nc.declare_dram_parameter("w", [D, 1], f32, isOutput=False)
    out_e = nc.declare_dram_parameter("out", [BL, TH, F], f32, isOutput=True)

    with tile.TileContext(nc) as tc:
        with (
            tc.tile_pool(name="consts", bufs=1) as consts,
            tc.tile_pool(name="sbig", bufs=3) as spool,
            tc.tile_pool(name="fbig", bufs=4) as fpool,
            tc.tile_pool(name="qred", bufs=2) as qredp,
            tc.tile_pool(name="smalls", bufs=2) as smalls,
            tc.tile_pool(name="ps_t", bufs=2, space="PSUM") as ps_t,
            tc.tile_pool(name="ps_p", bufs=2, space="PSUM") as ps_p,
            tc.tile_pool(name="ps_u", bufs=1, space="PSUM") as ps_u,
            tc.tile_pool(name="ps_q", bufs=3, space="PSUM") as ps_q,
        ):
            # ---------- load + prep ----------
            # Chunked loads: each HW DMA queue moves ~78 GB/s, so big tensors
            # are split across queues and casts run per-chunk as data lands.
            # v/h on sync, W/U on scalar, b/w on gpsimd (SWDGE).
            vf32 = consts.tile([128, BL, F], f32)
            hf32 = consts.tile([TH, BL, H], f32)
            Wf32 = consts.tile([128, FCN, D], f32)
            Uf32 = consts.tile([128, HCN, D], f32)
            nc.sync.dma_start(out=vf32[:, 0, 0:256], in_=v_e[0, :, 0:256])
            nc.sync.dma_start(out=vf32[:, 0, 256:512], in_=v_e[0, :, 256:512])
            nc.sync.dma_start(out=hf32[:, 0, :], in_=h_e[0])
            for fc in range(FCN):
                nc.scalar.dma_start(
                    out=Wf32[:, fc, :], in_=W_e[fc * 128 : (fc + 1) * 128, :]
                )
            for hc in range(HCN):
                nc.scalar.dma_start(
                    out=Uf32[:, hc, :], in_=U_e[hc * 128 : (hc + 1) * 128, :]
                )
            nc.sync.dma_start(out=vf32[:, 1, :], in_=v_e[1])
            nc.sync.dma_start(out=hf32[:, 1, :], in_=h_e[1])
            bsb = consts.tile([128, DCN], f32)
            nc.gpsimd.dma_start(out=bsb[:], in_=b_e[:].rearrange("(c p) -> p c", p=128))
            wsb = consts.tile([128, DCN], f32)
            nc.gpsimd.dma_start(out=wsb[:], in_=w_e[:, 0].rearrange("(c p) -> p c", p=128))

            # touch ACT early so the exp/tanh table set loads before the chain
            scrap = consts.tile([128, DCN], f32)
            nc.scalar.activation(scrap[:], bsb[:], AF.Tanh)

            vbf = consts.tile([128, BL, F], bf16)
            hbf = consts.tile([TH, BL, H], bf16)
            Wbf = consts.tile([128, FCN, D], bf16)
            Ubf = consts.tile([128, HCN, D], bf16)
            for fc in range(FCN):
                nc.vector.tensor_copy(
                    vbf[:, 0, fc * 128 : (fc + 1) * 128],
                    vf32[:, 0, fc * 128 : (fc + 1) * 128],
                )
                nc.vector.tensor_copy(Wbf[:, fc, :], Wf32[:, fc, :])
            for hc in range(HCN):
                nc.vector.tensor_copy(
                    hbf[:, 0, hc * 128 : (hc + 1) * 128],
                    hf32[:, 0, hc * 128 : (hc + 1) * 128],
                )
                nc.vector.tensor_copy(Ubf[:, hc, :], Uf32[:, hc, :])
            wbf = consts.tile([128, DCN], bf16)
            nc.vector.tensor_copy(wbf[:], wsb[:])
            w_rep = consts.tile([128, DCN, 32], bf16)
            for dc in range(DCN):
                nc.vector.tensor_copy(
                    w_rep[:, dc, :], wbf[:, dc : dc + 1].broadcast_to([128, 32])
                )

            ident = consts.tile([128, 128], bf16)
            make_identity(nc, ident)

            # transposes via PE (f32 in, bf16 out through the PSUM copyback)
            vT = consts.tile([128, BL, FCN, 128], bf16)   # [f_p, b, fc, t]
            hT = consts.tile([128, BL, HCN, TH], bf16)    # [h_p, b, hc, s]
            WvT = consts.tile([128, BL, DCN, TV], bf16)   # [d_p, b, dc, t]
            Uh2 = consts.tile([128, BL, DCN, TH, 2], bf16)  # [d_p, b, dc, s, dup]

            for b in range(BL):
                # ---------- transposes + projections ----------
                if b == 0:
                    # critical prefix: PE transposes interleaved with the
                    # accumulating projection matmuls (all bf16)
                    wv_ps = [ps_p.tile([128, 128], f32, tag="psp", name=f"wv_ps{dc}") for dc in range(DCN)]
                    for fc in range(FCN):
                        tp = ps_t.tile([128, 128], bf16, tag="pst")
                        nc.tensor.transpose(
                            tp[:], vbf[:, b, fc * 128 : (fc + 1) * 128], ident[:]
                        )
                        nc.vector.tensor_copy(vT[:, b, fc, :], tp[:])
                        for dc in range(DCN):
                            nc.tensor.matmul(
                                wv_ps[dc][:],
                                lhsT=Wbf[:, fc, dc * 128 : (dc + 1) * 128],
                                rhs=vT[:, b, fc, :],
                                start=(fc == 0),
                                stop=(fc == FCN - 1),
                            )
                    for dc in range(DCN):
                        nc.vector.tensor_copy(WvT[:, b, dc, :], wv_ps[dc][:])
                    uh_ps = [ps_p.tile([128, TH], f32, tag="psp", name=f"uh_ps{dc}") for dc in range(DCN)]
                    for hc in range(HCN):
                        tp = ps_t.tile([128, 128], bf16, tag="pst")
                        nc.tensor.transpose(
                            tp[:, :TH],
                            hbf[:, b, hc * 128 : (hc + 1) * 128],
                            ident[:TH, :TH],
                        )
                        nc.vector.tensor_copy(hT[:, b, hc, :], tp[:, :TH])
                        for dc in range(DCN):
                            nc.tensor.matmul(
                                uh_ps[dc][:],
                                lhsT=Ubf[:, hc, dc * 128 : (dc + 1) * 128],
                                rhs=hT[:, b, hc, :],
                                start=(hc == 0),
                                stop=(hc == HCN - 1),
                            )
                    for dc in range(DCN):
                        # duplicate each s value twice along free so the later
                        # tensor_tensor read has innermost step 1 (2x DVE mode)
                        nc.vector.tensor_copy(
                            Uh2[:, b, dc, :, :],
                            uh_ps[dc][:].unsqueeze(2).broadcast_to([128, TH, 2]),
                        )
                else:
                    # overlapped under batch-0 compute: xbar DMA transposes on
                    # the otherwise-idle sync engine
                    nc.vector.tensor_copy(vbf[:, b, :], vf32[:, b, :])
                    nc.vector.tensor_copy(hbf[:, b, :], hf32[:, b, :])
                    for fc in range(FCN):
                        nc.sync.dma_start_transpose(
                            vT[:, b, fc, :], vbf[:, b, fc * 128 : (fc + 1) * 128]
                        )
                    for hc in range(HCN):
                        nc.sync.dma_start_transpose(
                            hT[:, b, hc, :], hbf[:, b, hc * 128 : (hc + 1) * 128]
                        )
                    for dc in range(DCN):
                        dlo, dhi = dc * 128, (dc + 1) * 128
                        wv_ps0 = ps_p.tile([128, 128], f32, tag="psp")
                        for fc in range(FCN):
                            nc.tensor.matmul(
                                wv_ps0[:],
                                lhsT=Wbf[:, fc, dlo:dhi],
                                rhs=vT[:, b, fc, :],
                                start=(fc == 0),
                                stop=(fc == FCN - 1),
                            )
                        nc.vector.tensor_copy(WvT[:, b, dc, :], wv_ps0[:])
                        uh_ps0 = ps_p.tile([128, TH], f32, tag="psp")
                        for hc in range(HCN):
                            nc.tensor.matmul(
                                uh_ps0[:],
                                lhsT=Ubf[:, hc, dlo:dhi],
                                rhs=hT[:, b, hc, :],
                                start=(hc == 0),
                                stop=(hc == HCN - 1),
                            )
                        nc.vector.tensor_copy(
                            Uh2[:, b, dc, :, :],
                            uh_ps0[:].unsqueeze(2).broadcast_to([128, TH, 2]),
                        )

                # ---------- main: S build (DVE) -> tanh (ACT) -> w-dot (PE) ----
                # per s-half: col-group g covers s_local in [8g, 8g+8)
                for sh in range(2):  # s-halves of 32
                    f_tiles = {}
                    qps = [ps_q.tile([128, 512], f32, tag="qps", name=f"qps{R}")
                           for R in range(2)]
                    for dc in range(DCN):
                        s_t = spool.tile([128, 32, 128], bf16, tag="s")
                        in0 = WvT[:, b, dc, :].unsqueeze(1).broadcast_to([128, 32, 128])
                        in1 = (
                            Uh2[:, b, dc, sh * 32 : (sh + 1) * 32, :]
                            .unsqueeze(2)
                            .broadcast_to([128, 32, 64, 2])
                        )
                        nc.vector.tensor_add(s_t[:], in0, in1)
                        f_t = fpool.tile([128, 32, 128], bf16, tag="f")
                        nc.scalar.activation(
                            f_t[:], s_t[:], AF.Tanh, bias=bsb[:, dc : dc + 1], scale=1.0
                        )
                        f_tiles[dc] = f_t
                        # w-dot for this d-chunk: q[s,t] = sum_d w[d] f[d,s,t]
                        # s_local = 16R + 4g + s_sub  (R-blocks are contiguous)
                        for R in range(2):
                            for g in range(4):
                                so = 16 * R + 4 * g
                                nc.tensor.matmul(
                                    qps[R][32 * g : 32 * (g + 1), :],
                                    lhsT=w_rep[:, dc, :],
                                    rhs=f_t[:, so : so + 4, :],
                                    start=(dc == 0),
                                    stop=(dc == DCN - 1),
                                    tile_position=(0, 32 * g),
                                )
                    qred = qredp.tile([128, 2, 512], bf16, tag="qred")
                    eT = smalls.tile([128, 32], bf16, tag="eT")
                    for R in range(2):
                        rlo, rhi = 16 * R, 16 * (R + 1)
                        e_bf = smalls.tile([16, TV], bf16, tag="e", name=f"e{R}")
                        den = smalls.tile([16, 1], f32, tag="den", name=f"den{R}")
                        rden = smalls.tile([16, 1], f32, tag="rden", name=f"rden{R}")
                        q_sb = qredp.tile([16, TV], bf16, tag="qsb", name=f"qsb{R}")
                        nc.vector.tensor_copy(qred[:, R, :], qps[R][:])
                        # row 32g holds q for s_local 16R+[4g,4g+4): diagonal
                        nc.sync.dma_start(
                            out=q_sb[:],
                            in_=qred[::32, R, :].rearrange("g (s t) -> g s t", s=4),
                        )
                        # softmax over t with fused denominator accumulation
                        nc.scalar.activation(
                            e_bf[:], q_sb[:], AF.Exp,
                            bias=0.0, scale=1.0, accum_out=den[:],
                        )
                        nc.vector.reciprocal(rden[:], den[:])
                        # context: u = (e @ v) / den
                        btp = ps_t.tile([128, 16], bf16, tag="pst")
                        nc.tensor.transpose(btp[:], e_bf[:], ident[:16, :16])
                        nc.vector.tensor_copy(eT[:, rlo:rhi], btp[:])
                        ups = ps_u.tile([16, F], f32)
                        nc.tensor.matmul(
                            ups[:], lhsT=eT[:, rlo:rhi], rhs=vbf[:, b, :],
                            start=True, stop=True,
                        )
                        usb = smalls.tile([16, F], f32, tag="usb")
                        nc.vector.tensor_scalar_mul(usb[:], ups[:], rden[:])
                        nc.sync.dma_start(
                            out=out_e[b, sh * 32 + rlo : sh * 32 + rhi, :],
                            in_=usb[:],
                        )

    _split_excess_waits(nc, mybir)
    return nc


def _get_nc():
    if "nc" not in _CACHE:
        _CACHE["nc"] = _build_nc()
    return _CACHE["nc"]


def _in_maps(v, h, W, U, b, w):
    v = np.ascontiguousarray(np.asarray(v, dtype=np.float32))
    h = np.ascontiguousarray(np.asarray(h, dtype=np.float32))
    W = np.ascontiguousarray(np.asarray(W, dtype=np.float32))
    U = np.ascontiguousarray(np.asarray(U, dtype=np.float32))
    b = np.ascontiguousarray(np.asarray(b, dtype=np.float32))
    w = np.ascontiguousarray(np.asarray(w, dtype=np.float32))
    return [
        {
            "v": np.ascontiguousarray(v[i * BL : (i + 1) * BL]),
            "h": np.ascontiguousarray(h[i * BL : (i + 1) * BL]),
            "W": W,
            "U": U,
            "b": b,
            "w": w,
        }
        for i in range(NCORES)
    ]


def _run(in_maps, trace=False, tmpdir=None):
    from concourse.bass_utils import run_bass_kernel_spmd

    nc = _get_nc()
    return run_bass_kernel_spmd(
        nc, in_maps, core_ids=list(range(NCORES)), trace=trace, tmpdir=tmpdir
    )


def kernel(v, h, W, U, b, w):
    res = _run(_in_maps(v, h, W, U, b, w), trace=False)
    return np.concatenate([res.results[i]["out"] for i in range(NCORES)], axis=0)


def _install_ntff_hook():
    """The agent image's antenv lacks axon_hooks; recreate it so
    run_bass_kernel_spmd(trace=True) can NTFF-profile via the axon .so."""
    import sys
    import types

    try:
        from antenv.axon_hooks import get_axon_ntff_profile_hook  # noqa: F401
        return
    except ImportError:
        pass
    import antenv
    from trn_agent_boot.trn_boot import _ntff_profile_via_ctypes

    mod = types.ModuleType("antenv.axon_hooks")
    state = {"hook": _ntff_profile_via_ctypes("/opt/axon/libaxon_pjrt.so")}
    mod.set_axon_ntff_profile_hook = lambda h: state.__setitem__("hook", h)
    mod.get_axon_ntff_profile_hook = lambda: state["hook"]
    sys.modules["antenv.axon_hooks"] = mod
    antenv.axon_hooks = mod


def kernel_traced(v, h, W, U, b, w, tmpdir=None):
    """Returns (output, exec_time_ns) using the NTFF profile path."""
    _install_ntff_hook()
    import concourse.bass_utils as bu

    bu.upload_artifacts = lambda d: str(d)  # keep artifacts local
    res = _run(_in_maps(v, h, W, U, b, w), trace=True, tmpdir=tmpdir)
    out = np.concatenate([res.results[i]["out"] for i in range(NCORES)], axis=0)
    return out, res.exec_time_ns
